# revision 27
# baseline (speedup 1.0000x reference)
"""Trainium2 Bass kernel for the DNF (semi-symbolic dense MLP) problem.

Reference computation (per layer, x:(b,in), W:(out,in)):
    abs_w   = |x[:,i,None] * W.T[None,i,o]|          # (b, in, out)
    max_abs = max_i abs_w ; sum_abs = sum_i abs_w
    out     = x @ W.T + delta * (+/-)(max_abs - sum_abs)
Layer 1 (conjunction, +): tanh applied; layer 2 (disjunction, -).

Strategy: data-parallel over batch across 8 cores (128 rows each); weights
replicated.  All O(b*in*out) work runs on the TensorEngine:
  - x @ W.T and |x| @ |W|.T as float32r matmuls (1 cycle/row at N=512)
  - max_i |x_i||W_oi| via an even-power ratio-of-p-norms estimator:
        max^2 ~= sum_i (a_i c_i)^34 / sum_i (a_i c_i)^32
    computed as two bf16 matmuls over element-wise powered operands
    (each power = ONE fused custom-DVE op reading the transpose PSUM
    directly - even powers need no abs), followed by a Sqrt on the
    scalar engine.  The ratio form cancels rounding errors of the power
    factors: they only perturb the weights of a weighted mean over
    exact (a_i c_i)^2 terms.
"""

import math

import numpy as np

BATCH = 1024
NPRED = 512   # layer-1 contraction (in)
NCONJ = 512   # layer-1 out / layer-2 contraction
NOUT = 128    # layer-2 out
NCORES = 8
BSH = BATCH // NCORES  # 128 batch rows per core

W1SC = 3.0         # global scale for |W1| (keeps (s*c)^34 in range)
W2SC = 2.0         # global scale for |W2|
DELTA = 0.1

_CACHE = {}


def _register_pow_ops():
    """POW32S: (s0*x)^32; POW33S: (s0*x)^33 - fused squaring-chain DVE ops."""
    if "pow_ops" in _CACHE:
        return _CACHE["pow_ops"]
    import concourse.dve_ops as DO
    from concourse.dve_spec import Spec, Src0, C0, sq, lower
    from concourse.dve_spec import _has_src1 as has_src1
    from concourse.dve_uop import DveOpSpec

    def make(name, spec):
        opcode = DO._CUSTOM_DVE_ROW_BASE + len(DO.OPS)
        assert opcode < 0x20
        op = DO.DveOp(name, spec, subdim=False, uops_sha={})
        DO.OPS.append(op)
        DO._SUB_OPCODE_FOR_NAME[name] = opcode
        DO.CUSTOM_DVE_SPECS[name] = spec
        for ver in ("v3",):
            compiled = DveOpSpec(
                name=name, opcode=opcode,
                uops=lower(spec, ver=ver), rd1_en=has_src1(spec),
            )
            op.uops_sha[ver] = compiled.sha(ver)
        return op

    t = Src0 * C0
    pow32 = make(
        "POW32S_ANT",
        Spec(body=sq(sq(sq(sq(sq(t))))),
             reference=lambda in0, in1, c0, c1, c2: (
                 (np.float32(c0) * in0.astype(np.float32)) ** 32)),
    )
    t2 = Src0 * C0
    pow33 = make(
        "POW33S_ANT",
        Spec(body=sq(sq(sq(sq(sq(t2))))) * t2,
             reference=lambda in0, in1, c0, c1, c2: (
                 (np.float32(c0) * in0.astype(np.float32)) ** 33)),
    )
    _CACHE["pow_ops"] = (pow32, pow33)
    return pow32, pow33


def _build_nc():
    import concourse.mybir as mybir
    import concourse.tile as tile
    from concourse import bacc
    from concourse.tile import add_dep_helper

    fp32 = mybir.dt.float32
    f32r = mybir.dt.float32r
    bf16 = mybir.dt.bfloat16
    AF = mybir.ActivationFunctionType
    ALU = mybir.AluOpType

    POW32, POW33 = _register_pow_ops()

    nc = bacc.Bacc("TRN2", debug=False)

    x_d = nc.dram_tensor("x", (BSH, NPRED), fp32, kind="ExternalInput").ap()
    w1t_d = nc.dram_tensor("w1t", (NPRED // 128, 128, NCONJ), f32r,
                           kind="ExternalInput").ap()
    w2t_d = nc.dram_tensor("w2t", (NCONJ // 128, 128, NOUT), f32r,
                           kind="ExternalInput").ap()
    id_d = nc.dram_tensor("ident", (128, 128), fp32, kind="ExternalInput").ap()
    out_d = nc.dram_tensor("out", (BSH, NOUT), fp32, kind="ExternalOutput").ap()

    KC1 = NPRED // 128
    KC2 = NCONJ // 128

    def flat(t):
        return t.rearrange("p a b -> p (a b)")

    with tile.TileContext(nc) as tc:
        with (
            tc.tile_pool(name="const", bufs=1) as const_pool,
            tc.tile_pool(name="sb", bufs=1) as sb,
            tc.tile_pool(name="ptr", bufs=2, space="PSUM") as ptr,
            tc.tile_pool(name="pmm", bufs=4, space="PSUM") as pmm,
        ):
            # ---------------- input DMAs ----------------
            x_nat = sb.tile([128, NPRED], fp32, tag="x_nat")
            x_engs = (nc.sync, nc.scalar, nc.sync, nc.scalar)
            for h in range(4):
                x_engs[h].dma_start(out=x_nat[:, h * 128:(h + 1) * 128],
                                    in_=x_d[:, h * 128:(h + 1) * 128])
            ident = const_pool.tile([128, 128], fp32, tag="ident")
            nc.sync.dma_start(out=ident, in_=id_d)
            # pre-transposed weights, straight into their SBUF layouts
            w1T = sb.tile([128, KC1, NCONJ], f32r, tag="w1T")        # (i, o)
            w1_engs = {(0, 0): nc.scalar, (0, 1): nc.gpsimd,
                       (1, 0): nc.scalar, (1, 1): nc.gpsimd,
                       (2, 0): nc.scalar, (2, 1): nc.gpsimd,
                       (3, 0): nc.sync, (3, 1): nc.scalar}
            for ic in range(KC1):
                for h in range(2):
                    w1_engs[(ic, h)].dma_start(
                        out=w1T[:, ic, h * 256:(h + 1) * 256],
                        in_=w1t_d[ic, :, h * 256:(h + 1) * 256],
                    )
            w2T = sb.tile([128, KC2, NOUT], f32r, tag="w2T")         # (o, n)
            for oc in range(KC2):
                nc.gpsimd.dma_start(out=w2T[:, oc, :], in_=w2t_d[oc])

            # ---------------- x transposes + prep ----------------
            xT = sb.tile([128, KC1, 128], f32r, tag="xT")          # (i, b)
            xT_abs = sb.tile([128, KC1, 128], f32r, tag="xT_abs")  # 0.1|x|T
            fa = sb.tile([128, KC1, 128], bf16, tag="fa")          # x^32
            ga = sb.tile([128, KC1, 128], bf16, tag="ga")
            pt = ptr.tile([128, 512], fp32, tag="pt")
            for ic in range(KC1):
                nc.tensor.transpose(
                    pt[:, ic * 128:(ic + 1) * 128],
                    x_nat[:, ic * 128:(ic + 1) * 128],
                    ident,
                )
            i_cp_x = nc.scalar.activation(flat(xT), pt, AF.Copy)
            i_abs_x = nc.scalar.activation(flat(xT_abs), pt, AF.Abs, scale=DELTA)
            nc.vector._custom_dve(POW32, out=flat(fa), in0=pt, s0=1.0)
            nc.vector._custom_dve(
                POW33, out=flat(ga), in0=flat(xT_abs).bitcast(fp32),
                s0=(DELTA / W1SC) ** (1.0 / 33) / DELTA)

            # ---------------- w2 prep (from DMA-loaded w2T) ------------
            w2T_abs = sb.tile([128, KC2, NOUT], fp32, tag="w2T_abs")
            fc2 = sb.tile([128, KC2, NOUT], bf16, tag="fc2")       # (s2 c)^32
            gc2 = sb.tile([128, KC2, NOUT], bf16, tag="gc2")       # (s2 c)^33
            i_abs_w2 = nc.scalar.activation(flat(w2T_abs),
                                            flat(w2T).bitcast(fp32), AF.Abs,
                                            scale=DELTA)

            # ---------------- w1 prep (from DMA-loaded w1T) ------------
            w1T_abs = sb.tile([128, KC1, NCONJ], f32r, tag="w1T_abs")
            fc1 = sb.tile([128, KC1, NCONJ], bf16, tag="fc1")
            gc1 = sb.tile([128, KC1, NCONJ], bf16, tag="gc1")
            act_chain = []
            for ic in range(KC1):
                act_chain.append(
                    nc.scalar.activation(w1T_abs[:, ic, :],
                                         w1T[:, ic, :].bitcast(fp32), AF.Abs))
                nc.vector._custom_dve(POW32, out=fc1[:, ic, :],
                                      in0=w1T[:, ic, :].bitcast(fp32),
                                      s0=W1SC)
                nc.vector._custom_dve(
                    POW33, out=gc1[:, ic, :],
                    in0=w1T_abs[:, ic, :].bitcast(fp32), s0=W1SC)

            # ---------------- layer-1 matmuls (out = (b, o)) -----------
            mm1 = pmm.tile([128, NCONJ], fp32, tag="mmpsum")  # x @ W1.T
            s1 = pmm.tile([128, NCONJ], fp32, tag="mmpsum")   # 0.1|x| @ |W1|.T
            sp1 = pmm.tile([128, NCONJ], fp32, tag="mmpsum")
            sq1 = pmm.tile([128, NCONJ], fp32, tag="mmpsum")
            for psum, xt, wt in (
                (mm1, xT, w1T),
                (s1, xT_abs, w1T_abs),
                (sp1, fa, fc1),
                (sq1, ga, gc1),
            ):
                for ic in range(KC1):
                    nc.tensor.matmul(
                        psum, xt[:, ic, :], wt[:, ic, :],
                        start=(ic == 0), stop=(ic == KC1 - 1),
                    )

            # w2 estimator powers (needed only for layer 2 - low priority)
            nc.vector._custom_dve(POW32, out=flat(fc2),
                                  in0=flat(w2T).bitcast(fp32), s0=W2SC)
            nc.vector._custom_dve(POW33, out=flat(gc2), in0=flat(w2T_abs),
                                  s0=W2SC / DELTA)

            # ---------------- layer-1 epilogue ----------------
            # z = mm1 - s1 runs while the estimator matmuls still stream
            mm1_sb = sb.tile([128, NCONJ], fp32, tag="mm1_sb")
            i_cp_mm1 = nc.scalar.activation(mm1_sb, mm1, AF.Copy)
            z1 = sb.tile([128, NCONJ], fp32, tag="z1")
            nc.vector.tensor_tensor(out=z1, in0=s1, in1=mm1_sb,
                                    op=ALU.subtract)  # s1 - mm1 = -(mm1-s1)
            rp1 = sb.tile([128, NCONJ], fp32, tag="rp1")
            nc.vector.reciprocal_approx_fast(out=rp1, in_=sp1)
            tq1 = sb.tile([128, NCONJ], fp32, tag="tq1")   # 0.1 * max1
            nc.vector.tensor_tensor(out=tq1, in0=sq1, in1=rp1, op=ALU.mult)
            v2 = sb.tile([128, NCONJ], fp32, tag="v2")     # z1 - tq1 = -conj_
            nc.vector.tensor_tensor(out=v2, in0=z1, in1=tq1, op=ALU.subtract)
            conj = sb.tile([128, NCONJ], fp32, tag="conj")
            i_tanh = nc.scalar.activation(conj, v2, AF.Tanh, scale=-1.0)

            # ---------------- conj transpose + prep ----------------
            conjT = sb.tile([128, KC2, 128], f32r, tag="conjT")      # (o, b)
            cT_abs = sb.tile([128, KC2, 128], fp32, tag="cT_abs")    # |c|T
            fa2 = sb.tile([128, KC2, 128], bf16, tag="fa2")          # c^32
            ga2 = sb.tile([128, KC2, 128], bf16, tag="ga2")
            ptc = ptr.tile([128, 512], fp32, tag="pt")
            for oc in range(KC2):
                nc.tensor.transpose(
                    ptc[:, oc * 128:(oc + 1) * 128],
                    conj[:, oc * 128:(oc + 1) * 128],
                    ident,
                )
            nc.vector.tensor_copy(flat(conjT), ptc)
            u32 = mybir.dt.uint32
            nc.vector.tensor_scalar(
                flat(cT_abs).bitcast(u32), ptc.bitcast(u32),
                0x7FFFFFFF, None, ALU.bitwise_and)
            nc.vector._custom_dve(POW32, out=flat(fa2), in0=ptc, s0=1.0)
            nc.vector._custom_dve(
                POW33, out=flat(ga2), in0=flat(cT_abs),
                s0=(DELTA * W2SC ** 32) ** (1.0 / 33) / W2SC)

            # ---------------- layer-2 matmuls ----------------
            mm2 = pmm.tile([128, NOUT], fp32, tag="mmpsum")
            s2 = pmm.tile([128, NOUT], fp32, tag="mmpsum")
            sp2 = pmm.tile([128, NOUT], fp32, tag="mmpsum")
            sq2 = pmm.tile([128, NOUT], fp32, tag="mmpsum")
            for psum, ct, wt in (
                (mm2, conjT, w2T),
                (s2, cT_abs, w2T_abs),
                (sp2, fa2, fc2),
                (sq2, ga2, gc2),
            ):
                for oc in range(KC2):
                    nc.tensor.matmul(
                        psum, ct[:, oc, :], wt[:, oc, :],
                        start=(oc == 0), stop=(oc == KC2 - 1),
                    )

            # ---------------- layer-2 epilogue ----------------
            rp2 = sb.tile([128, NOUT], fp32, tag="rp2")
            nc.vector.reciprocal_approx_fast(out=rp2, in_=sp2)
            tq2 = sb.tile([128, NOUT], fp32, tag="tq2")    # 0.1 * max2
            nc.vector.tensor_tensor(out=tq2, in0=sq2, in1=rp2, op=ALU.mult)
            u1 = sb.tile([128, NOUT], fp32, tag="u1")      # 0.1*S2 - 0.1*max2
            nc.vector.tensor_tensor(out=u1, in0=s2, in1=tq2, op=ALU.subtract)
            res = sb.tile([128, NOUT], fp32, tag="res")
            nc.vector.tensor_tensor(out=res, in0=mm2, in1=u1, op=ALU.add)
            nc.sync.dma_start(out=out_d, in_=res)

            # scalar-engine ordering (stable tables / no thrash)
            act_chain += [i_abs_w2, i_cp_x, i_abs_x, i_cp_mm1, i_tanh]
            for prev, nxt in zip(act_chain, act_chain[1:]):
                add_dep_helper(nxt.ins, prev.ins, sync=False,
                               reason="act order")

    nc.compile()
    return nc


def _get_nc():
    if "nc" not in _CACHE:
        _CACHE["nc"] = _build_nc()
    return _CACHE["nc"]


_IDENT = np.eye(128, dtype=np.float32)


def kernel(x: np.ndarray, W_conj: np.ndarray, W_disj: np.ndarray) -> np.ndarray:
    from concourse.bass_utils import run_bass_kernel_spmd

    x = np.ascontiguousarray(x, dtype=np.float32)
    W_conj = np.ascontiguousarray(W_conj, dtype=np.float32)
    W_disj = np.ascontiguousarray(W_disj, dtype=np.float32)

    nc = _get_nc()
    w1t = np.ascontiguousarray(W_conj.T).reshape(NPRED // 128, 128, NCONJ)
    w2t = np.ascontiguousarray(W_disj.T).reshape(NCONJ // 128, 128, NOUT)
    in_maps = [
        {
            "x": x[c * BSH:(c + 1) * BSH],
            "w1t": w1t,
            "w2t": w2t,
            "ident": _IDENT,
        }
        for c in range(NCORES)
    ]
    res = run_bass_kernel_spmd(nc, in_maps, core_ids=list(range(NCORES)))
    return np.concatenate([r["out"] for r in res.results], axis=0)


# revision 28
# speedup vs baseline: 1.0203x; 1.0203x over previous
"""Trainium2 Bass kernel for the DNF (semi-symbolic dense MLP) problem.

Reference computation (per layer, x:(b,in), W:(out,in)):
    abs_w   = |x[:,i,None] * W.T[None,i,o]|          # (b, in, out)
    max_abs = max_i abs_w ; sum_abs = sum_i abs_w
    out     = x @ W.T + delta * (+/-)(max_abs - sum_abs)
Layer 1 (conjunction, +): tanh applied; layer 2 (disjunction, -).

Strategy: data-parallel over batch across 8 cores (128 rows each); weights
replicated.  All O(b*in*out) work runs on the TensorEngine:
  - x @ W.T and |x| @ |W|.T as float32r matmuls (1 cycle/row at N=512)
  - max_i |x_i||W_oi| via an even-power ratio-of-p-norms estimator:
        max^2 ~= sum_i (a_i c_i)^34 / sum_i (a_i c_i)^32
    computed as two bf16 matmuls over element-wise powered operands
    (each power = ONE fused custom-DVE op reading the transpose PSUM
    directly - even powers need no abs), followed by a Sqrt on the
    scalar engine.  The ratio form cancels rounding errors of the power
    factors: they only perturb the weights of a weighted mean over
    exact (a_i c_i)^2 terms.
"""

import math

import numpy as np

BATCH = 1024
NPRED = 512   # layer-1 contraction (in)
NCONJ = 512   # layer-1 out / layer-2 contraction
NOUT = 128    # layer-2 out
NCORES = 8
BSH = BATCH // NCORES  # 128 batch rows per core

W1SC = 3.0         # global scale for |W1| (keeps (s*c)^34 in range)
W2SC = 2.0         # global scale for |W2|
DELTA = 0.1

_CACHE = {}


def _register_pow_ops():
    """POW32S: (s0*x)^32; POW33S: (s0*x)^33 - fused squaring-chain DVE ops."""
    if "pow_ops" in _CACHE:
        return _CACHE["pow_ops"]
    import concourse.dve_ops as DO
    from concourse.dve_spec import Spec, Src0, C0, sq, lower
    from concourse.dve_spec import _has_src1 as has_src1
    from concourse.dve_uop import DveOpSpec

    def make(name, spec):
        opcode = DO._CUSTOM_DVE_ROW_BASE + len(DO.OPS)
        assert opcode < 0x20
        op = DO.DveOp(name, spec, subdim=False, uops_sha={})
        DO.OPS.append(op)
        DO._SUB_OPCODE_FOR_NAME[name] = opcode
        DO.CUSTOM_DVE_SPECS[name] = spec
        for ver in ("v3",):
            compiled = DveOpSpec(
                name=name, opcode=opcode,
                uops=lower(spec, ver=ver), rd1_en=has_src1(spec),
            )
            op.uops_sha[ver] = compiled.sha(ver)
        return op

    t = Src0 * C0
    pow32 = make(
        "POW32S_ANT",
        Spec(body=sq(sq(sq(sq(sq(t))))),
             reference=lambda in0, in1, c0, c1, c2: (
                 (np.float32(c0) * in0.astype(np.float32)) ** 32)),
    )
    t2 = Src0 * C0
    pow33 = make(
        "POW33S_ANT",
        Spec(body=sq(sq(sq(sq(sq(t2))))) * t2,
             reference=lambda in0, in1, c0, c1, c2: (
                 (np.float32(c0) * in0.astype(np.float32)) ** 33)),
    )
    _CACHE["pow_ops"] = (pow32, pow33)
    return pow32, pow33


def _build_nc():
    import concourse.mybir as mybir
    import concourse.tile as tile
    from concourse import bacc
    from concourse.tile import add_dep_helper

    fp32 = mybir.dt.float32
    f32r = mybir.dt.float32r
    bf16 = mybir.dt.bfloat16
    AF = mybir.ActivationFunctionType
    ALU = mybir.AluOpType

    POW32, POW33 = _register_pow_ops()

    nc = bacc.Bacc("TRN2", debug=False)

    x_d = nc.dram_tensor("x", (BSH, NPRED), fp32, kind="ExternalInput").ap()
    w1t_d = nc.dram_tensor("w1t", (NPRED // 128, 128, NCONJ), f32r,
                           kind="ExternalInput").ap()
    w2t_d = nc.dram_tensor("w2t", (NCONJ // 128, 128, NOUT), f32r,
                           kind="ExternalInput").ap()
    id_d = nc.dram_tensor("ident", (128, 128), fp32, kind="ExternalInput").ap()
    out_d = nc.dram_tensor("out", (BSH, NOUT), fp32, kind="ExternalOutput").ap()

    KC1 = NPRED // 128
    KC2 = NCONJ // 128

    def flat(t):
        return t.rearrange("p a b -> p (a b)")

    with tile.TileContext(nc) as tc:
        with (
            tc.tile_pool(name="const", bufs=1) as const_pool,
            tc.tile_pool(name="sb", bufs=1) as sb,
            tc.tile_pool(name="ptr", bufs=2, space="PSUM") as ptr,
            tc.tile_pool(name="pmm", bufs=4, space="PSUM") as pmm,
        ):
            # ---------------- input DMAs ----------------
            x_nat = sb.tile([128, NPRED], fp32, tag="x_nat")
            x_engs = (nc.sync, nc.scalar, nc.sync, nc.scalar)
            for h in range(4):
                x_engs[h].dma_start(out=x_nat[:, h * 128:(h + 1) * 128],
                                    in_=x_d[:, h * 128:(h + 1) * 128])
            ident = const_pool.tile([128, 128], fp32, tag="ident")
            nc.sync.dma_start(out=ident, in_=id_d)
            # pre-transposed weights, straight into their SBUF layouts
            w1T = sb.tile([128, KC1, NCONJ], f32r, tag="w1T")        # (i, o)
            w1_engs = {(0, 0): nc.scalar, (0, 1): nc.gpsimd,
                       (1, 0): nc.scalar, (1, 1): nc.gpsimd,
                       (2, 0): nc.scalar, (2, 1): nc.gpsimd,
                       (3, 0): nc.sync, (3, 1): nc.scalar}
            for ic in range(KC1):
                for h in range(2):
                    w1_engs[(ic, h)].dma_start(
                        out=w1T[:, ic, h * 256:(h + 1) * 256],
                        in_=w1t_d[ic, :, h * 256:(h + 1) * 256],
                    )
            w2T = sb.tile([128, KC2, NOUT], f32r, tag="w2T")         # (o, n)
            for oc in range(KC2):
                nc.gpsimd.dma_start(out=w2T[:, oc, :], in_=w2t_d[oc])

            # ---------------- x transposes + prep ----------------
            xT = sb.tile([128, KC1, 128], f32r, tag="xT")          # (i, b)
            xT_abs = sb.tile([128, KC1, 128], f32r, tag="xT_abs")  # 0.1|x|T
            fa = sb.tile([128, KC1, 128], bf16, tag="fa")          # x^32
            ga = sb.tile([128, KC1, 128], bf16, tag="ga")
            pt = ptr.tile([128, 512], fp32, tag="pt")
            for ic in range(KC1):
                nc.tensor.transpose(
                    pt[:, ic * 128:(ic + 1) * 128],
                    x_nat[:, ic * 128:(ic + 1) * 128],
                    ident,
                )
            i_cp_x = nc.scalar.activation(flat(xT), pt, AF.Copy)
            i_abs_x = nc.scalar.activation(flat(xT_abs), pt, AF.Abs, scale=DELTA)
            nc.vector._custom_dve(POW32, out=flat(fa), in0=pt, s0=1.0)
            nc.vector._custom_dve(
                POW33, out=flat(ga), in0=flat(xT_abs).bitcast(fp32),
                s0=(DELTA / W1SC) ** (1.0 / 33) / DELTA)

            # ---------------- w2 prep (from DMA-loaded w2T) ------------
            w2T_abs = sb.tile([128, KC2, NOUT], fp32, tag="w2T_abs")
            fc2 = sb.tile([128, KC2, NOUT], bf16, tag="fc2")       # (s2 c)^32
            gc2 = sb.tile([128, KC2, NOUT], bf16, tag="gc2")       # (s2 c)^33
            i_abs_w2 = nc.scalar.activation(flat(w2T_abs),
                                            flat(w2T).bitcast(fp32), AF.Abs,
                                            scale=DELTA)

            # ---------------- w1 prep (from DMA-loaded w1T) ------------
            w1T_abs = sb.tile([128, KC1, NCONJ], f32r, tag="w1T_abs")
            fc1 = sb.tile([128, KC1, NCONJ], bf16, tag="fc1")
            gc1 = sb.tile([128, KC1, NCONJ], bf16, tag="gc1")
            act_chain = [i_cp_x, i_abs_x, i_abs_w2]
            for ic in range(KC1):
                act_chain.append(
                    nc.scalar.activation(w1T_abs[:, ic, :],
                                         w1T[:, ic, :].bitcast(fp32), AF.Abs))
                nc.vector._custom_dve(POW32, out=fc1[:, ic, :],
                                      in0=w1T[:, ic, :].bitcast(fp32),
                                      s0=W1SC)
                nc.vector._custom_dve(
                    POW33, out=gc1[:, ic, :],
                    in0=w1T_abs[:, ic, :].bitcast(fp32), s0=W1SC)

            # ---------------- layer-1 matmuls (out = (b, o)) -----------
            mm1 = pmm.tile([128, NCONJ], fp32, tag="mmpsum")  # x @ W1.T
            s1 = pmm.tile([128, NCONJ], fp32, tag="mmpsum")   # 0.1|x| @ |W1|.T
            sp1 = pmm.tile([128, NCONJ], fp32, tag="mmpsum")
            sq1 = pmm.tile([128, NCONJ], fp32, tag="mmpsum")
            for psum, xt, wt in (
                (mm1, xT, w1T),
                (s1, xT_abs, w1T_abs),
                (sp1, fa, fc1),
                (sq1, ga, gc1),
            ):
                for ic in range(KC1):
                    nc.tensor.matmul(
                        psum, xt[:, ic, :], wt[:, ic, :],
                        start=(ic == 0), stop=(ic == KC1 - 1),
                    )

            # w2 estimator powers (needed only for layer 2 - low priority)
            nc.vector._custom_dve(POW32, out=flat(fc2),
                                  in0=flat(w2T).bitcast(fp32), s0=W2SC)
            nc.vector._custom_dve(POW33, out=flat(gc2), in0=flat(w2T_abs),
                                  s0=W2SC / DELTA)

            # ---------------- layer-1 epilogue ----------------
            # z = mm1 - s1 runs while the estimator matmuls still stream
            mm1_sb = sb.tile([128, NCONJ], fp32, tag="mm1_sb")
            i_cp_mm1 = nc.scalar.activation(mm1_sb, mm1, AF.Copy)
            z1 = sb.tile([128, NCONJ], fp32, tag="z1")
            nc.vector.tensor_tensor(out=z1, in0=s1, in1=mm1_sb,
                                    op=ALU.subtract)  # s1 - mm1 = -(mm1-s1)
            rp1 = sb.tile([128, NCONJ], fp32, tag="rp1")
            nc.vector.reciprocal_approx_fast(out=rp1, in_=sp1)
            tq1 = sb.tile([128, NCONJ], fp32, tag="tq1")   # 0.1 * max1
            nc.vector.tensor_tensor(out=tq1, in0=sq1, in1=rp1, op=ALU.mult)
            v2 = sb.tile([128, NCONJ], fp32, tag="v2")     # z1 - tq1 = -conj_
            nc.vector.tensor_tensor(out=v2, in0=z1, in1=tq1, op=ALU.subtract)
            conj = sb.tile([128, NCONJ], fp32, tag="conj")
            i_tanh = nc.scalar.activation(conj, v2, AF.Tanh, scale=-1.0)

            # ---------------- conj transpose + prep ----------------
            conjT = sb.tile([128, KC2, 128], f32r, tag="conjT")      # (o, b)
            cT_abs = sb.tile([128, KC2, 128], fp32, tag="cT_abs")    # |c|T
            fa2 = sb.tile([128, KC2, 128], bf16, tag="fa2")          # c^32
            ga2 = sb.tile([128, KC2, 128], bf16, tag="ga2")
            ptc = ptr.tile([128, 512], fp32, tag="pt")
            for oc in range(KC2):
                nc.tensor.transpose(
                    ptc[:, oc * 128:(oc + 1) * 128],
                    conj[:, oc * 128:(oc + 1) * 128],
                    ident,
                )
            nc.vector.tensor_copy(flat(conjT), ptc)
            u32 = mybir.dt.uint32
            nc.vector.tensor_scalar(
                flat(cT_abs).bitcast(u32), ptc.bitcast(u32),
                0x7FFFFFFF, None, ALU.bitwise_and)
            nc.vector._custom_dve(POW32, out=flat(fa2), in0=ptc, s0=1.0)
            nc.vector._custom_dve(
                POW33, out=flat(ga2), in0=flat(cT_abs),
                s0=(DELTA * W2SC ** 32) ** (1.0 / 33) / W2SC)

            # ---------------- layer-2 matmuls ----------------
            mm2 = pmm.tile([128, NOUT], fp32, tag="mmpsum")
            s2 = pmm.tile([128, NOUT], fp32, tag="mmpsum")
            sp2 = pmm.tile([128, NOUT], fp32, tag="mmpsum")
            sq2 = pmm.tile([128, NOUT], fp32, tag="mmpsum")
            for psum, ct, wt in (
                (mm2, conjT, w2T),
                (s2, cT_abs, w2T_abs),
                (sp2, fa2, fc2),
                (sq2, ga2, gc2),
            ):
                for oc in range(KC2):
                    nc.tensor.matmul(
                        psum, ct[:, oc, :], wt[:, oc, :],
                        start=(oc == 0), stop=(oc == KC2 - 1),
                    )

            # ---------------- layer-2 epilogue ----------------
            rp2 = sb.tile([128, NOUT], fp32, tag="rp2")
            nc.vector.reciprocal_approx_fast(out=rp2, in_=sp2)
            tq2 = sb.tile([128, NOUT], fp32, tag="tq2")    # 0.1 * max2
            nc.vector.tensor_tensor(out=tq2, in0=sq2, in1=rp2, op=ALU.mult)
            u1 = sb.tile([128, NOUT], fp32, tag="u1")      # 0.1*S2 - 0.1*max2
            nc.vector.tensor_tensor(out=u1, in0=s2, in1=tq2, op=ALU.subtract)
            res = sb.tile([128, NOUT], fp32, tag="res")
            nc.vector.tensor_tensor(out=res, in0=mm2, in1=u1, op=ALU.add)
            nc.sync.dma_start(out=out_d, in_=res)

            # scalar-engine ordering (stable tables / no thrash)
            act_chain += [i_cp_mm1, i_tanh]
            for prev, nxt in zip(act_chain, act_chain[1:]):
                add_dep_helper(nxt.ins, prev.ins, sync=False,
                               reason="act order")

    nc.compile()
    return nc


def _get_nc():
    if "nc" not in _CACHE:
        _CACHE["nc"] = _build_nc()
    return _CACHE["nc"]


_IDENT = np.eye(128, dtype=np.float32)


def kernel(x: np.ndarray, W_conj: np.ndarray, W_disj: np.ndarray) -> np.ndarray:
    from concourse.bass_utils import run_bass_kernel_spmd

    x = np.ascontiguousarray(x, dtype=np.float32)
    W_conj = np.ascontiguousarray(W_conj, dtype=np.float32)
    W_disj = np.ascontiguousarray(W_disj, dtype=np.float32)

    nc = _get_nc()
    w1t = np.ascontiguousarray(W_conj.T).reshape(NPRED // 128, 128, NCONJ)
    w2t = np.ascontiguousarray(W_disj.T).reshape(NCONJ // 128, 128, NOUT)
    in_maps = [
        {
            "x": x[c * BSH:(c + 1) * BSH],
            "w1t": w1t,
            "w2t": w2t,
            "ident": _IDENT,
        }
        for c in range(NCORES)
    ]
    res = run_bass_kernel_spmd(nc, in_maps, core_ids=list(range(NCORES)))
    return np.concatenate([r["out"] for r in res.results], axis=0)


# revision 30
# speedup vs baseline: 1.0320x; 1.0115x over previous
"""Trainium2 Bass kernel for the DNF (semi-symbolic dense MLP) problem.

Reference computation (per layer, x:(b,in), W:(out,in)):
    abs_w   = |x[:,i,None] * W.T[None,i,o]|          # (b, in, out)
    max_abs = max_i abs_w ; sum_abs = sum_i abs_w
    out     = x @ W.T + delta * (+/-)(max_abs - sum_abs)
Layer 1 (conjunction, +): tanh applied; layer 2 (disjunction, -).

Strategy: data-parallel over batch across 8 cores (128 rows each); weights
replicated.  All O(b*in*out) work runs on the TensorEngine:
  - x @ W.T and |x| @ |W|.T as float32r matmuls (1 cycle/row at N=512)
  - max_i |x_i||W_oi| via an even-power ratio-of-p-norms estimator:
        max^2 ~= sum_i (a_i c_i)^34 / sum_i (a_i c_i)^32
    computed as two bf16 matmuls over element-wise powered operands
    (each power = ONE fused custom-DVE op reading the transpose PSUM
    directly - even powers need no abs), followed by a Sqrt on the
    scalar engine.  The ratio form cancels rounding errors of the power
    factors: they only perturb the weights of a weighted mean over
    exact (a_i c_i)^2 terms.
"""

import math

import numpy as np

BATCH = 1024
NPRED = 512   # layer-1 contraction (in)
NCONJ = 512   # layer-1 out / layer-2 contraction
NOUT = 128    # layer-2 out
NCORES = 8
BSH = BATCH // NCORES  # 128 batch rows per core

W1SC = 3.0         # global scale for |W1| (keeps (s*c)^34 in range)
W2SC = 2.0         # global scale for |W2|
DELTA = 0.1

_CACHE = {}


def _register_pow_ops():
    """POW32S: (s0*x)^32; POW33S: (s0*x)^33 - fused squaring-chain DVE ops."""
    if "pow_ops" in _CACHE:
        return _CACHE["pow_ops"]
    import concourse.dve_ops as DO
    from concourse.dve_spec import Spec, Src0, C0, sq, lower
    from concourse.dve_spec import _has_src1 as has_src1
    from concourse.dve_uop import DveOpSpec

    def make(name, spec):
        for prev in DO.OPS:
            if prev.name == name:  # already registered (re-import)
                return prev
        opcode = DO._CUSTOM_DVE_ROW_BASE + len(DO.OPS)
        assert opcode < 0x20
        op = DO.DveOp(name, spec, subdim=False, uops_sha={})
        DO.OPS.append(op)
        DO._SUB_OPCODE_FOR_NAME[name] = opcode
        DO.CUSTOM_DVE_SPECS[name] = spec
        for ver in ("v3",):
            compiled = DveOpSpec(
                name=name, opcode=opcode,
                uops=lower(spec, ver=ver), rd1_en=has_src1(spec),
            )
            op.uops_sha[ver] = compiled.sha(ver)
        return op

    t = Src0 * C0
    pow32 = make(
        "POW32S_ANT",
        Spec(body=sq(sq(sq(sq(sq(t))))),
             reference=lambda in0, in1, c0, c1, c2: (
                 (np.float32(c0) * in0.astype(np.float32)) ** 32)),
    )
    t2 = Src0 * C0
    pow33 = make(
        "POW33S_ANT",
        Spec(body=sq(sq(sq(sq(sq(t2))))) * t2,
             reference=lambda in0, in1, c0, c1, c2: (
                 (np.float32(c0) * in0.astype(np.float32)) ** 33)),
    )
    _CACHE["pow_ops"] = (pow32, pow33)
    return pow32, pow33


def _build_nc():
    import concourse.mybir as mybir
    import concourse.tile as tile
    from concourse import bacc
    from concourse.tile import add_dep_helper

    fp32 = mybir.dt.float32
    f32r = mybir.dt.float32r
    bf16 = mybir.dt.bfloat16
    AF = mybir.ActivationFunctionType
    ALU = mybir.AluOpType

    POW32, POW33 = _register_pow_ops()

    nc = bacc.Bacc("TRN2", debug=False)

    x_d = nc.dram_tensor("x", (BSH, NPRED), fp32, kind="ExternalInput").ap()
    w1t_d = nc.dram_tensor("w1t", (NPRED // 128, 128, NCONJ), f32r,
                           kind="ExternalInput").ap()
    w2t_d = nc.dram_tensor("w2t", (NCONJ // 128, 128, NOUT), f32r,
                           kind="ExternalInput").ap()
    id_d = nc.dram_tensor("ident", (128, 128), fp32, kind="ExternalInput").ap()
    out_d = nc.dram_tensor("out", (BSH, NOUT), fp32, kind="ExternalOutput").ap()

    KC1 = NPRED // 128
    KC2 = NCONJ // 128

    def flat(t):
        return t.rearrange("p a b -> p (a b)")

    with tile.TileContext(nc) as tc:
        with (
            tc.tile_pool(name="const", bufs=1) as const_pool,
            tc.tile_pool(name="sb", bufs=1) as sb,
            tc.tile_pool(name="ptr", bufs=2, space="PSUM") as ptr,
            tc.tile_pool(name="pmm", bufs=4, space="PSUM") as pmm,
        ):
            # ---------------- input DMAs ----------------
            ident = const_pool.tile([128, 128], fp32, tag="ident")
            nc.sync.dma_start(out=ident, in_=id_d)
            x_nat = sb.tile([128, NPRED], fp32, tag="x_nat")
            x_engs = (nc.sync, nc.scalar, nc.sync, nc.scalar)
            for h in range(4):
                x_engs[h].dma_start(out=x_nat[:, h * 128:(h + 1) * 128],
                                    in_=x_d[:, h * 128:(h + 1) * 128])
            # pre-transposed weights, straight into their SBUF layouts
            w1T = sb.tile([128, KC1, NCONJ], f32r, tag="w1T")        # (i, o)
            w1_engs = {(0, 0): nc.scalar, (0, 1): nc.gpsimd,
                       (1, 0): nc.scalar, (1, 1): nc.gpsimd,
                       (2, 0): nc.scalar, (2, 1): nc.gpsimd,
                       (3, 0): nc.sync, (3, 1): nc.scalar}
            for ic in range(KC1):
                for h in range(2):
                    w1_engs[(ic, h)].dma_start(
                        out=w1T[:, ic, h * 256:(h + 1) * 256],
                        in_=w1t_d[ic, :, h * 256:(h + 1) * 256],
                    )
            w2T = sb.tile([128, KC2, NOUT], f32r, tag="w2T")         # (o, n)
            for oc in range(KC2):
                nc.gpsimd.dma_start(out=w2T[:, oc, :], in_=w2t_d[oc])

            # ---------------- x transposes + prep ----------------
            xT = sb.tile([128, KC1, 128], f32r, tag="xT")          # (i, b)
            xT_abs = sb.tile([128, KC1, 128], f32r, tag="xT_abs")  # 0.1|x|T
            fa = sb.tile([128, KC1, 128], bf16, tag="fa")          # x^32
            ga = sb.tile([128, KC1, 128], bf16, tag="ga")
            pt = ptr.tile([128, 512], fp32, tag="pt")
            for ic in range(KC1):
                nc.tensor.transpose(
                    pt[:, ic * 128:(ic + 1) * 128],
                    x_nat[:, ic * 128:(ic + 1) * 128],
                    ident,
                )
            i_cp_x = nc.scalar.activation(flat(xT), pt, AF.Copy)
            i_abs_x = nc.scalar.activation(flat(xT_abs), pt, AF.Abs, scale=DELTA)
            nc.vector._custom_dve(POW32, out=flat(fa), in0=pt, s0=1.0)
            nc.vector._custom_dve(
                POW33, out=flat(ga), in0=flat(xT_abs).bitcast(fp32),
                s0=(DELTA / W1SC) ** (1.0 / 33) / DELTA)

            # ---------------- w2 prep (from DMA-loaded w2T) ------------
            w2T_abs = sb.tile([128, KC2, NOUT], fp32, tag="w2T_abs")
            fc2 = sb.tile([128, KC2, NOUT], bf16, tag="fc2")       # (s2 c)^32
            gc2 = sb.tile([128, KC2, NOUT], bf16, tag="gc2")       # (s2 c)^33
            i_abs_w2 = nc.scalar.activation(flat(w2T_abs),
                                            flat(w2T).bitcast(fp32), AF.Abs,
                                            scale=DELTA)

            # ---------------- w1 prep (from DMA-loaded w1T) ------------
            w1T_abs = sb.tile([128, KC1, NCONJ], f32r, tag="w1T_abs")
            fc1 = sb.tile([128, KC1, NCONJ], bf16, tag="fc1")
            gc1 = sb.tile([128, KC1, NCONJ], bf16, tag="gc1")
            act_chain = [i_cp_x, i_abs_x, i_abs_w2]
            for ic in range(KC1):
                act_chain.append(
                    nc.scalar.activation(w1T_abs[:, ic, :],
                                         w1T[:, ic, :].bitcast(fp32), AF.Abs))
                nc.vector._custom_dve(POW32, out=fc1[:, ic, :],
                                      in0=w1T[:, ic, :].bitcast(fp32),
                                      s0=W1SC)
                nc.vector._custom_dve(
                    POW33, out=gc1[:, ic, :],
                    in0=w1T_abs[:, ic, :].bitcast(fp32), s0=W1SC)

            # ---------------- layer-1 matmuls (out = (b, o)) -----------
            mm1 = pmm.tile([128, NCONJ], fp32, tag="mmpsum")  # x @ W1.T
            s1 = pmm.tile([128, NCONJ], fp32, tag="mmpsum")   # 0.1|x| @ |W1|.T
            sp1 = pmm.tile([128, NCONJ], fp32, tag="mmpsum")
            sq1 = pmm.tile([128, NCONJ], fp32, tag="mmpsum")
            for psum, xt, wt in (
                (mm1, xT, w1T),
                (s1, xT_abs, w1T_abs),
                (sp1, fa, fc1),
                (sq1, ga, gc1),
            ):
                for ic in range(KC1):
                    nc.tensor.matmul(
                        psum, xt[:, ic, :], wt[:, ic, :],
                        start=(ic == 0), stop=(ic == KC1 - 1),
                    )

            # w2 estimator powers (needed only for layer 2 - low priority)
            nc.vector._custom_dve(POW32, out=flat(fc2),
                                  in0=flat(w2T).bitcast(fp32), s0=W2SC)
            nc.vector._custom_dve(POW33, out=flat(gc2), in0=flat(w2T_abs),
                                  s0=W2SC / DELTA)

            # ---------------- layer-1 epilogue ----------------
            # z = mm1 - s1 runs while the estimator matmuls still stream
            mm1_sb = sb.tile([128, NCONJ], fp32, tag="mm1_sb")
            i_cp_mm1 = nc.scalar.activation(mm1_sb, mm1, AF.Copy)
            z1 = sb.tile([128, NCONJ], fp32, tag="z1")
            nc.vector.tensor_tensor(out=z1, in0=s1, in1=mm1_sb,
                                    op=ALU.subtract)  # s1 - mm1 = -(mm1-s1)
            rp1 = sb.tile([128, NCONJ], fp32, tag="rp1")
            nc.vector.reciprocal_approx_fast(out=rp1, in_=sp1)
            tq1 = sb.tile([128, NCONJ], fp32, tag="tq1")   # 0.1 * max1
            nc.vector.tensor_tensor(out=tq1, in0=sq1, in1=rp1, op=ALU.mult)
            v2 = sb.tile([128, NCONJ], fp32, tag="v2")     # z1 - tq1 = -conj_
            nc.vector.tensor_tensor(out=v2, in0=z1, in1=tq1, op=ALU.subtract)
            conj = sb.tile([128, NCONJ], fp32, tag="conj")
            i_tanh = nc.scalar.activation(conj, v2, AF.Tanh, scale=-1.0)

            # ---------------- conj transpose + prep ----------------
            conjT = sb.tile([128, KC2, 128], f32r, tag="conjT")      # (o, b)
            cT_abs = sb.tile([128, KC2, 128], fp32, tag="cT_abs")    # |c|T
            fa2 = sb.tile([128, KC2, 128], bf16, tag="fa2")          # c^32
            ga2 = sb.tile([128, KC2, 128], bf16, tag="ga2")
            ptc = ptr.tile([128, 512], fp32, tag="pt")
            for oc in range(KC2):
                nc.tensor.transpose(
                    ptc[:, oc * 128:(oc + 1) * 128],
                    conj[:, oc * 128:(oc + 1) * 128],
                    ident,
                )
            nc.vector.tensor_copy(flat(conjT), ptc)
            u32 = mybir.dt.uint32
            nc.vector.tensor_scalar(
                flat(cT_abs).bitcast(u32), ptc.bitcast(u32),
                0x7FFFFFFF, None, ALU.bitwise_and)
            nc.vector._custom_dve(POW32, out=flat(fa2), in0=ptc, s0=1.0)
            nc.vector._custom_dve(
                POW33, out=flat(ga2), in0=flat(cT_abs),
                s0=(DELTA * W2SC ** 32) ** (1.0 / 33) / W2SC)

            # ---------------- layer-2 matmuls ----------------
            mm2 = pmm.tile([128, NOUT], fp32, tag="mmpsum")
            s2 = pmm.tile([128, NOUT], fp32, tag="mmpsum")
            sp2 = pmm.tile([128, NOUT], fp32, tag="mmpsum")
            sq2 = pmm.tile([128, NOUT], fp32, tag="mmpsum")
            for psum, ct, wt in (
                (mm2, conjT, w2T),
                (s2, cT_abs, w2T_abs),
                (sp2, fa2, fc2),
                (sq2, ga2, gc2),
            ):
                for oc in range(KC2):
                    nc.tensor.matmul(
                        psum, ct[:, oc, :], wt[:, oc, :],
                        start=(oc == 0), stop=(oc == KC2 - 1),
                    )

            # ---------------- layer-2 epilogue ----------------
            rp2 = sb.tile([128, NOUT], fp32, tag="rp2")
            nc.vector.reciprocal_approx_fast(out=rp2, in_=sp2)
            tq2 = sb.tile([128, NOUT], fp32, tag="tq2")    # 0.1 * max2
            nc.vector.tensor_tensor(out=tq2, in0=sq2, in1=rp2, op=ALU.mult)
            u1 = sb.tile([128, NOUT], fp32, tag="u1")      # 0.1*S2 - 0.1*max2
            nc.vector.tensor_tensor(out=u1, in0=s2, in1=tq2, op=ALU.subtract)
            res = sb.tile([128, NOUT], fp32, tag="res")
            nc.vector.tensor_tensor(out=res, in0=mm2, in1=u1, op=ALU.add)
            nc.sync.dma_start(out=out_d, in_=res)

            # scalar-engine ordering (stable tables / no thrash)
            act_chain += [i_cp_mm1, i_tanh]
            for prev, nxt in zip(act_chain, act_chain[1:]):
                add_dep_helper(nxt.ins, prev.ins, sync=False,
                               reason="act order")

    nc.compile()
    return nc


def _get_nc():
    if "nc" not in _CACHE:
        _CACHE["nc"] = _build_nc()
    return _CACHE["nc"]


_IDENT = np.eye(128, dtype=np.float32)


def kernel(x: np.ndarray, W_conj: np.ndarray, W_disj: np.ndarray) -> np.ndarray:
    from concourse.bass_utils import run_bass_kernel_spmd

    x = np.ascontiguousarray(x, dtype=np.float32)
    W_conj = np.ascontiguousarray(W_conj, dtype=np.float32)
    W_disj = np.ascontiguousarray(W_disj, dtype=np.float32)

    nc = _get_nc()
    w1t = np.ascontiguousarray(W_conj.T).reshape(NPRED // 128, 128, NCONJ)
    w2t = np.ascontiguousarray(W_disj.T).reshape(NCONJ // 128, 128, NOUT)
    in_maps = [
        {
            "x": x[c * BSH:(c + 1) * BSH],
            "w1t": w1t,
            "w2t": w2t,
            "ident": _IDENT,
        }
        for c in range(NCORES)
    ]
    res = run_bass_kernel_spmd(nc, in_maps, core_ids=list(range(NCORES)))
    return np.concatenate([r["out"] for r in res.results], axis=0)
